# revision 22
# baseline (speedup 1.0000x reference)
"""MoE top-2 (Switch-style) expert-parallel kernel for Trainium2, 8 NeuronCores.

Model dims (hardcoded from the problem spec):
  x:[B=8,S=512,D=512], w_gate:[D,E=8], W1:[E,D,H=1024], b1:[E,H],
  W2:[E,H,D], b2:[E,D], top-k K=2, mask:[B,S] in {0,1}.

Strategy (expert-parallel, matching the sharding hint):
  - Host computes the gating (logits -> top-2 -> softmax -> dense gates,
    masked) and dispatches: for each expert e, gather the tokens with a
    non-zero gate for e into a capacity-C buffer, transposed to [D, C]
    so the device never has to transpose activations.
  - Each of the 8 cores runs one expert's 2-layer MLP on its [D, C]
    token block, entirely in "transposed" layout:
        hT[H,C]   = relu(W1e^T-form matmul: lhsT=W1e[D,H], rhs=xeT[D,C]) + b1
        outT[D,C] = (lhsT=W2e[H,D], rhs=hT[H,C]) + b2
    fp32 data, fp32r matmuls (full PE rate at N>=256).
  - Host combines: y = x + sum_e gate_e * outT_e^T scattered back to the
    token positions (exactly equal to the dense reference formulation,
    since non-top-2 gates are exactly zero).

Codegen quirk this kernel works around: the walrus build here allows only
ONE sync-wait on Matmult/Activation/DMA instruction structs and ~8 on the
kernel-tail Drain. Hence: one DMA per input tensor (few HWDGE lanes),
tiny "absorber" ops that soak up DMA-completion waits, and the store on
the SWDGE (gpsimd) queue.
"""

import math

import numpy as np

B, S, D, H, E, TOPK = 8, 512, 512, 1024, 8, 2
N_CORES = 8
P = 128
C_DEFAULT = 640  # per-expert token capacity; max routed count is 545 for the fixed seed

LAST_RESULTS = None  # BassKernelResults of the most recent device run (for test.py)

_nc_cache: dict[tuple, object] = {}


def _chunks(C: int) -> list[tuple[int, int]]:
    """Split the free dim C into matmul chunks <=512, preferring >=256 so
    fp32r runs at full rate."""
    n = math.ceil(C / 512)
    base = C // n
    rem = C - base * n
    out = []
    off = 0
    for i in range(n):
        sz = base + (1 if i < rem else 0)
        out.append((off, sz))
        off += sz
    return out


def _build(C: int, dtype_mode: str = "f32r"):
    import concourse.bass as bass
    import concourse.mybir as mybir
    import concourse.tile as tile

    f32 = mybir.dt.float32
    # matmul operand dtype: float32r runs the PE at full rate (vs 4 cyc/row
    # for float32); producers of fp32r-matmul operands must also be fp32r.
    mm_dt = {"f32r": mybir.dt.float32r, "f32": f32}[dtype_mode]

    KD = D // P  # 4  k-tiles for layer 1 (contraction over D)
    KH = H // P  # 8  k-tiles for layer 2 (contraction over H)

    nc = bass.Bass("TRN2", target_bir_lowering=False, debug=False, num_devices=N_CORES)
    xeT_d = nc.dram_tensor("xeT", [D, C], mm_dt, kind="ExternalInput")
    w1_d = nc.dram_tensor("w1", [D, H], mm_dt, kind="ExternalInput")
    w2_d = nc.dram_tensor("w2", [H, D], mm_dt, kind="ExternalInput")
    b12_d = nc.dram_tensor("b12c", [P, KH + KD], f32, kind="ExternalInput")
    out_d = nc.dram_tensor("outT", [D, C], f32, kind="ExternalOutput")

    relu = mybir.ActivationFunctionType.Relu
    ident = mybir.ActivationFunctionType.Identity
    cspans = _chunks(C)

    with tile.TileContext(nc) as tc:
        with (
            tc.tile_pool(name="sb", bufs=1) as sb,
            tc.tile_pool(name="ps", bufs=8, space="PSUM") as ps,
        ):
            w1_t = [sb.tile([P, H], mm_dt, tag=f"w1_{k}", name=f"w1_{k}") for k in range(KD)]
            xe_t = [sb.tile([P, C], mm_dt, tag=f"xe_{k}", name=f"xe_{k}") for k in range(KD)]
            w2_t = [sb.tile([P, D], mm_dt, tag=f"w2_{k}", name=f"w2_{k}") for k in range(KH)]
            b12_t = sb.tile([P, KH + KD], f32)
            hT_t = [sb.tile([P, C], mm_dt, tag=f"h_{k}", name=f"h_{k}") for k in range(KH)]
            out_t = [sb.tile([P, C], f32, tag=f"o_{k}", name=f"o_{k}") for k in range(KD)]

            # Loads ride three concurrent DMA streams (each physical ring is
            # ~200 GB/s; HBM caps the sum at ~360): SP HWDGE ring gets W1,
            # ACT HWDGE ring gets xeT+biases, SWDGE (gpsimd) gets W2.
            # Layer-1 operands are per-k slabs so its matmuls start early.
            w1_r = w1_d.ap().rearrange("(ko p) h -> p ko h", p=P)
            xe_r = xeT_d.ap().rearrange("(ko p) c -> p ko c", p=P)
            w2_r = w2_d.ap().rearrange("(ko p) d -> p ko d", p=P)
            for k in range(KD):
                nc.sync.dma_start(w1_t[k][:], w1_r[:, k])
                nc.scalar.dma_start(xe_t[k][:], xe_r[:, k])
            nc.scalar.dma_start(b12_t[:], b12_d.ap())
            for k in range(KH):
                nc.gpsimd.dma_start(w2_t[k][:], w2_r[:, k])

            # layer 1: hT[hi] = relu(sum_k W1[k,hi]^T @ xeT[k] + b1[hi])
            for hi in range(KH):
                for c0, cw in cspans:
                    pt = ps.tile([P, cw], f32, tag="ps")
                    for k in range(KD):
                        nc.tensor.matmul(
                            pt[:],
                            w1_t[k][:, hi * P : (hi + 1) * P],
                            xe_t[k][:, c0 : c0 + cw],
                            start=(k == 0),
                            stop=(k == KD - 1),
                        )
                    nc.scalar.activation(
                        hT_t[hi][:, c0 : c0 + cw], pt[:], relu, bias=b12_t[:, hi : hi + 1]
                    )

            # layer 2: outT[di] = sum_k W2[k,di]^T @ hT[k] + b2[di]
            for di in range(KD):
                for c0, cw in cspans:
                    pt = ps.tile([P, cw], f32, tag="ps")
                    for k in range(KH):
                        nc.tensor.matmul(
                            pt[:],
                            w2_t[k][:, di * P : (di + 1) * P],
                            hT_t[k][:, c0 : c0 + cw],
                            start=(k == 0),
                            stop=(k == KH - 1),
                        )
                    nc.scalar.activation(
                        out_t[di][:, c0 : c0 + cw],
                        pt[:],
                        ident,
                        bias=b12_t[:, KH + di : KH + di + 1],
                    )

            # per-slab stores on the ACT ring: producer (activation) is the
            # same engine, so stores need no semaphores and overlap L2
            o_r = out_d.ap().rearrange("(ko p) c -> p ko c", p=P)
            for k in range(KD):
                nc.scalar.dma_start(o_r[:, k], out_t[k][:])

    _split_multi_waits(nc, mybir)
    return nc


def _split_multi_waits(nc, mybir):
    """This walrus build allows only one sync-wait per engine instruction.
    Split any multi-wait instruction: hoist all but the last wait onto
    single-wait NoOps inserted just before it on the same engine."""
    n = 0
    for f in nc.m.functions:
        for blk in f.blocks:
            insts = blk.instructions
            i = 0
            while i < len(insts):
                inst = insts[i]
                si = inst.sync_info
                if si is not None and len(si.on_wait) > 1:
                    waits = list(si.on_wait)
                    for j, w in enumerate(waits[:-1]):
                        nop = mybir.InstNoOp(
                            name=f"ant-waitsplit-{n}",
                            engine=inst.engine,
                            ins=[],
                            outs=[],
                            sync_info=mybir.SyncInfo(on_wait=[w], on_update=[]),
                        )
                        n += 1
                        insts.insert(i, nop)
                        i += 1
                    inst.sync_info = mybir.SyncInfo(
                        on_wait=[waits[-1]], on_update=list(si.on_update)
                    )
                i += 1


def _get_nc(C: int, dtype_mode: str):
    key = (C, dtype_mode)
    if key not in _nc_cache:
        _nc_cache[key] = _build(C, dtype_mode)
    return _nc_cache[key]


def _route(xf: np.ndarray, mask_f: np.ndarray, w_gate: np.ndarray):
    """Top-2 gating on host. Returns per-expert (positions, gate values)."""
    N = xf.shape[0]
    logits = xf @ w_gate  # [N, E] f32
    rows = np.arange(N)
    i1 = np.argmax(logits, axis=1)
    v1 = logits[rows, i1]
    l2 = logits.copy()
    l2[rows, i1] = -np.inf
    i2 = np.argmax(l2, axis=1)
    v2 = l2[rows, i2]
    # softmax over the two top values (v1 >= v2)
    e2 = np.exp(v2 - v1)
    s = 1.0 + e2
    g1 = (1.0 / s).astype(np.float32)
    g2 = (e2 / s).astype(np.float32)
    active = mask_f != 0
    pos, gv = [], []
    for e in range(E):
        s1 = (i1 == e) & active
        s2 = (i2 == e) & active
        p = np.concatenate([np.nonzero(s1)[0], np.nonzero(s2)[0]])
        g = np.concatenate([g1[s1], g2[s2]])
        pos.append(p)
        gv.append(g)
    return pos, gv


def kernel(x, mask, w_gate, W1, b1, W2, b2, dtype_mode="f32r", trace=False):
    global LAST_RESULTS
    from concourse.bass_utils import run_bass_kernel_spmd

    x = np.asarray(x, dtype=np.float32)
    mask_f = np.asarray(mask).reshape(-1)
    w_gate = np.asarray(w_gate, dtype=np.float32)
    W1 = np.asarray(W1, dtype=np.float32)
    b1 = np.asarray(b1, dtype=np.float32)
    W2 = np.asarray(W2, dtype=np.float32)
    b2 = np.asarray(b2, dtype=np.float32)

    xf = x.reshape(-1, D)
    pos, gv = _route(xf, mask_f, w_gate)
    maxc = max(len(p) for p in pos)
    C = max(C_DEFAULT, ((maxc + P - 1) // P) * P)

    nc = _get_nc(C, dtype_mode)

    in_maps = []
    for e in range(E):
        xeT = np.zeros((D, C), dtype=np.float32)
        n_e = len(pos[e])
        if n_e:
            xeT[:, :n_e] = xf[pos[e]].T
        b12 = np.concatenate(
            [b1[e].reshape(H // P, P).T, b2[e].reshape(D // P, P).T], axis=1
        )
        in_maps.append(
            {
                "xeT": xeT,
                "w1": np.ascontiguousarray(W1[e]),
                "w2": np.ascontiguousarray(W2[e]),
                "b12c": np.ascontiguousarray(b12),
            }
        )

    res = run_bass_kernel_spmd(nc, in_maps, core_ids=list(range(N_CORES)), trace=trace)
    LAST_RESULTS = res

    y = xf.copy()
    for e in range(E):
        n_e = len(pos[e])
        if n_e:
            y[pos[e]] += gv[e][:, None] * res.results[e]["outT"][:, :n_e].T
    return y.reshape(B, S, D)


# revision 23
# speedup vs baseline: 1.0274x; 1.0274x over previous
"""MoE top-2 (Switch-style) expert-parallel kernel for Trainium2, 8 NeuronCores.

Model dims (hardcoded from the problem spec):
  x:[B=8,S=512,D=512], w_gate:[D,E=8], W1:[E,D,H=1024], b1:[E,H],
  W2:[E,H,D], b2:[E,D], top-k K=2, mask:[B,S] in {0,1}.

Strategy (expert-parallel, matching the sharding hint):
  - Host computes the gating (logits -> top-2 -> softmax -> dense gates,
    masked) and dispatches: for each expert e, gather the tokens with a
    non-zero gate for e into a capacity-C buffer, transposed to [D, C]
    so the device never has to transpose activations.
  - Each of the 8 cores runs one expert's 2-layer MLP on its [D, C]
    token block, entirely in "transposed" layout:
        hT[H,C]   = relu(W1e^T-form matmul: lhsT=W1e[D,H], rhs=xeT[D,C]) + b1
        outT[D,C] = (lhsT=W2e[H,D], rhs=hT[H,C]) + b2
    fp32 data, fp32r matmuls (full PE rate at N>=256).
  - Host combines: y = x + sum_e gate_e * outT_e^T scattered back to the
    token positions (exactly equal to the dense reference formulation,
    since non-top-2 gates are exactly zero).

Codegen quirk this kernel works around: the walrus build here allows only
ONE sync-wait on Matmult/Activation/DMA instruction structs and ~8 on the
kernel-tail Drain. Hence: one DMA per input tensor (few HWDGE lanes),
tiny "absorber" ops that soak up DMA-completion waits, and the store on
the SWDGE (gpsimd) queue.
"""

import math

import numpy as np

B, S, D, H, E, TOPK = 8, 512, 512, 1024, 8, 2
N_CORES = 8
P = 128
C_DEFAULT = 640  # per-expert token capacity; max routed count is 545 for the fixed seed

LAST_RESULTS = None  # BassKernelResults of the most recent device run (for test.py)

_nc_cache: dict[tuple, object] = {}


def _chunks(C: int) -> list[tuple[int, int]]:
    """Split the free dim C into matmul chunks <=512, preferring >=256 so
    fp32r runs at full rate."""
    n = math.ceil(C / 512)
    base = C // n
    rem = C - base * n
    out = []
    off = 0
    for i in range(n):
        sz = base + (1 if i < rem else 0)
        out.append((off, sz))
        off += sz
    return out


def _build(C: int, dtype_mode: str = "f32r"):
    import concourse.bass as bass
    import concourse.mybir as mybir
    import concourse.tile as tile

    f32 = mybir.dt.float32
    # matmul operand dtype. float32r streams at the fp32 byte rate (~2
    # cycles/row); bfloat16 streams at 1 cycle/row and halves DMA bytes.
    mm_dt = {
        "f32r": mybir.dt.float32r,
        "f32": f32,
        "bf16": mybir.dt.bfloat16,
    }[dtype_mode]

    KD = D // P  # 4  k-tiles for layer 1 (contraction over D)
    KH = H // P  # 8  k-tiles for layer 2 (contraction over H)

    nc = bass.Bass("TRN2", target_bir_lowering=False, debug=False, num_devices=N_CORES)
    xeT_d = nc.dram_tensor("xeT", [D, C], mm_dt, kind="ExternalInput")
    w1_d = nc.dram_tensor("w1", [D, H], mm_dt, kind="ExternalInput")
    w2_d = nc.dram_tensor("w2", [H, D], mm_dt, kind="ExternalInput")
    b12_d = nc.dram_tensor("b12c", [P, KH + KD], f32, kind="ExternalInput")
    out_d = nc.dram_tensor("outT", [D, C], f32, kind="ExternalOutput")

    relu = mybir.ActivationFunctionType.Relu
    ident = mybir.ActivationFunctionType.Identity
    cspans = _chunks(C)

    with tile.TileContext(nc) as tc:
        with (
            tc.tile_pool(name="sb", bufs=1) as sb,
            tc.tile_pool(name="ps", bufs=8, space="PSUM") as ps,
        ):
            w1_t = [sb.tile([P, H], mm_dt, tag=f"w1_{k}", name=f"w1_{k}") for k in range(KD)]
            xe_t = [sb.tile([P, C], mm_dt, tag=f"xe_{k}", name=f"xe_{k}") for k in range(KD)]
            w2_t = [sb.tile([P, D], mm_dt, tag=f"w2_{k}", name=f"w2_{k}") for k in range(KH)]
            b12_t = sb.tile([P, KH + KD], f32)
            hT_t = [sb.tile([P, C], mm_dt, tag=f"h_{k}", name=f"h_{k}") for k in range(KH)]
            out_t = [sb.tile([P, C], f32, tag=f"o_{k}", name=f"o_{k}") for k in range(KD)]

            # Loads ride three concurrent DMA streams (each physical ring is
            # ~200 GB/s; HBM caps the sum at ~360): SP HWDGE ring gets W1,
            # ACT HWDGE ring gets xeT+biases, SWDGE (gpsimd) gets W2.
            # Layer-1 operands are per-k slabs so its matmuls start early.
            w1_r = w1_d.ap().rearrange("(ko p) h -> p ko h", p=P)
            xe_r = xeT_d.ap().rearrange("(ko p) c -> p ko c", p=P)
            w2_r = w2_d.ap().rearrange("(ko p) d -> p ko d", p=P)
            for k in range(KD):
                nc.sync.dma_start(w1_t[k][:], w1_r[:, k])
                nc.scalar.dma_start(xe_t[k][:], xe_r[:, k])
            nc.scalar.dma_start(b12_t[:], b12_d.ap())
            for k in range(KH):
                nc.gpsimd.dma_start(w2_t[k][:], w2_r[:, k])

            # layer 1: hT[hi] = relu(sum_k W1[k,hi]^T @ xeT[k] + b1[hi])
            for hi in range(KH):
                for c0, cw in cspans:
                    pt = ps.tile([P, cw], f32, tag="ps")
                    for k in range(KD):
                        nc.tensor.matmul(
                            pt[:],
                            w1_t[k][:, hi * P : (hi + 1) * P],
                            xe_t[k][:, c0 : c0 + cw],
                            start=(k == 0),
                            stop=(k == KD - 1),
                        )
                    nc.scalar.activation(
                        hT_t[hi][:, c0 : c0 + cw], pt[:], relu, bias=b12_t[:, hi : hi + 1]
                    )

            # layer 2: outT[di] = sum_k W2[k,di]^T @ hT[k] + b2[di]
            for di in range(KD):
                for c0, cw in cspans:
                    pt = ps.tile([P, cw], f32, tag="ps")
                    for k in range(KH):
                        nc.tensor.matmul(
                            pt[:],
                            w2_t[k][:, di * P : (di + 1) * P],
                            hT_t[k][:, c0 : c0 + cw],
                            start=(k == 0),
                            stop=(k == KH - 1),
                        )
                    nc.scalar.activation(
                        out_t[di][:, c0 : c0 + cw],
                        pt[:],
                        ident,
                        bias=b12_t[:, KH + di : KH + di + 1],
                    )

            # per-slab stores on the ACT ring: producer (activation) is the
            # same engine, so stores need no semaphores and overlap L2
            o_r = out_d.ap().rearrange("(ko p) c -> p ko c", p=P)
            for k in range(KD):
                nc.scalar.dma_start(o_r[:, k], out_t[k][:])

    _split_multi_waits(nc, mybir)
    return nc


def _split_multi_waits(nc, mybir):
    """This walrus build allows only one sync-wait per engine instruction.
    Split any multi-wait instruction: hoist all but the last wait onto
    single-wait NoOps inserted just before it on the same engine."""
    n = 0
    for f in nc.m.functions:
        for blk in f.blocks:
            insts = blk.instructions
            i = 0
            while i < len(insts):
                inst = insts[i]
                si = inst.sync_info
                if si is not None and len(si.on_wait) > 1:
                    waits = list(si.on_wait)
                    for j, w in enumerate(waits[:-1]):
                        nop = mybir.InstNoOp(
                            name=f"ant-waitsplit-{n}",
                            engine=inst.engine,
                            ins=[],
                            outs=[],
                            sync_info=mybir.SyncInfo(on_wait=[w], on_update=[]),
                        )
                        n += 1
                        insts.insert(i, nop)
                        i += 1
                    inst.sync_info = mybir.SyncInfo(
                        on_wait=[waits[-1]], on_update=list(si.on_update)
                    )
                i += 1


def _get_nc(C: int, dtype_mode: str):
    key = (C, dtype_mode)
    if key not in _nc_cache:
        _nc_cache[key] = _build(C, dtype_mode)
    return _nc_cache[key]


def _route(xf: np.ndarray, mask_f: np.ndarray, w_gate: np.ndarray):
    """Top-2 gating on host. Returns per-expert (positions, gate values)."""
    N = xf.shape[0]
    logits = xf @ w_gate  # [N, E] f32
    rows = np.arange(N)
    i1 = np.argmax(logits, axis=1)
    v1 = logits[rows, i1]
    l2 = logits.copy()
    l2[rows, i1] = -np.inf
    i2 = np.argmax(l2, axis=1)
    v2 = l2[rows, i2]
    # softmax over the two top values (v1 >= v2)
    e2 = np.exp(v2 - v1)
    s = 1.0 + e2
    g1 = (1.0 / s).astype(np.float32)
    g2 = (e2 / s).astype(np.float32)
    active = mask_f != 0
    pos, gv = [], []
    for e in range(E):
        s1 = (i1 == e) & active
        s2 = (i2 == e) & active
        p = np.concatenate([np.nonzero(s1)[0], np.nonzero(s2)[0]])
        g = np.concatenate([g1[s1], g2[s2]])
        pos.append(p)
        gv.append(g)
    return pos, gv


def kernel(x, mask, w_gate, W1, b1, W2, b2, dtype_mode="f32r", trace=False):
    global LAST_RESULTS
    import ml_dtypes
    from concourse.bass_utils import run_bass_kernel_spmd

    in_np_dt = ml_dtypes.bfloat16 if dtype_mode == "bf16" else np.float32

    x = np.asarray(x, dtype=np.float32)
    mask_f = np.asarray(mask).reshape(-1)
    w_gate = np.asarray(w_gate, dtype=np.float32)
    W1 = np.asarray(W1, dtype=np.float32)
    b1 = np.asarray(b1, dtype=np.float32)
    W2 = np.asarray(W2, dtype=np.float32)
    b2 = np.asarray(b2, dtype=np.float32)

    xf = x.reshape(-1, D)
    pos, gv = _route(xf, mask_f, w_gate)
    maxc = max(len(p) for p in pos)
    C = max(C_DEFAULT, ((maxc + P - 1) // P) * P)

    nc = _get_nc(C, dtype_mode)

    in_maps = []
    for e in range(E):
        xeT = np.zeros((D, C), dtype=in_np_dt)
        n_e = len(pos[e])
        if n_e:
            xeT[:, :n_e] = xf[pos[e]].T.astype(in_np_dt)
        b12 = np.concatenate(
            [b1[e].reshape(H // P, P).T, b2[e].reshape(D // P, P).T], axis=1
        )
        in_maps.append(
            {
                "xeT": xeT,
                "w1": np.ascontiguousarray(W1[e].astype(in_np_dt)),
                "w2": np.ascontiguousarray(W2[e].astype(in_np_dt)),
                "b12c": np.ascontiguousarray(b12),
            }
        )

    res = run_bass_kernel_spmd(nc, in_maps, core_ids=list(range(N_CORES)), trace=trace)
    LAST_RESULTS = res

    y = xf.copy()
    for e in range(E):
        n_e = len(pos[e])
        if n_e:
            y[pos[e]] += gv[e][:, None] * res.results[e]["outT"][:, :n_e].T
    return y.reshape(B, S, D)


# revision 25
# speedup vs baseline: 1.0743x; 1.0457x over previous
"""MoE top-2 (Switch-style) expert-parallel kernel for Trainium2, 8 NeuronCores.

Model dims (hardcoded from the problem spec):
  x:[B=8,S=512,D=512], w_gate:[D,E=8], W1:[E,D,H=1024], b1:[E,H],
  W2:[E,H,D], b2:[E,D], top-k K=2, mask:[B,S] in {0,1}.

Strategy (expert-parallel, matching the sharding hint):
  - Host computes the gating (logits -> top-2 -> softmax -> dense gates,
    masked) and dispatches: for each expert e, gather the tokens with a
    non-zero gate for e into a capacity-C buffer, transposed to [D, C]
    so the device never has to transpose activations.
  - Each of the 8 cores runs one expert's 2-layer MLP on its [D, C]
    token block, entirely in "transposed" layout:
        hT[H,C]   = relu(W1e^T-form matmul: lhsT=W1e[D,H], rhs=xeT[D,C]) + b1
        outT[D,C] = (lhsT=W2e[H,D], rhs=hT[H,C]) + b2
    fp32 data, fp32r matmuls (full PE rate at N>=256).
  - Host combines: y = x + sum_e gate_e * outT_e^T scattered back to the
    token positions (exactly equal to the dense reference formulation,
    since non-top-2 gates are exactly zero).

Codegen quirk this kernel works around: the walrus build here allows only
ONE sync-wait on Matmult/Activation/DMA instruction structs and ~8 on the
kernel-tail Drain. Hence: one DMA per input tensor (few HWDGE lanes),
tiny "absorber" ops that soak up DMA-completion waits, and the store on
the SWDGE (gpsimd) queue.
"""

import math

import numpy as np

B, S, D, H, E, TOPK = 8, 512, 512, 1024, 8, 2
N_CORES = 8
P = 128
C_DEFAULT = 640  # per-expert token capacity; max routed count is 545 for the fixed seed

LAST_RESULTS = None  # BassKernelResults of the most recent device run (for test.py)

_nc_cache: dict[tuple, object] = {}


def _chunks(C: int) -> list[tuple[int, int]]:
    """Split the free dim C into matmul chunks <=512, preferring >=256 so
    fp32r runs at full rate."""
    n = math.ceil(C / 512)
    base = C // n
    rem = C - base * n
    out = []
    off = 0
    for i in range(n):
        sz = base + (1 if i < rem else 0)
        out.append((off, sz))
        off += sz
    return out


def _build(C: int, dtype_mode: str = "f32r"):
    import concourse.bass as bass
    import concourse.mybir as mybir
    import concourse.tile as tile

    f32 = mybir.dt.float32
    # matmul operand dtype. float32r streams at the fp32 byte rate (~2
    # cycles/row); bfloat16 streams at 1 cycle/row and halves DMA bytes.
    mm_dt = {
        "f32r": mybir.dt.float32r,
        "f32": f32,
        "bf16": mybir.dt.bfloat16,
    }[dtype_mode]

    KD = D // P  # 4  k-tiles for layer 1 (contraction over D)
    KH = H // P  # 8  k-tiles for layer 2 (contraction over H)

    nc = bass.Bass("TRN2", target_bir_lowering=False, debug=False, num_devices=N_CORES)
    # all inputs arrive pre-tiled partition-major: [P, ktiles*freedim], so
    # each load is one DMA with multi-KB contiguous per-partition lines
    xeT_d = nc.dram_tensor("xeT", [P, KD * C], mm_dt, kind="ExternalInput")
    w1_d = nc.dram_tensor("w1", [P, KD * H], mm_dt, kind="ExternalInput")
    w2_d = nc.dram_tensor("w2", [P, KH * D], mm_dt, kind="ExternalInput")
    b12_d = nc.dram_tensor("b12c", [P, KH + KD], f32, kind="ExternalInput")
    out_d = nc.dram_tensor("outT", [P, KD * C], f32, kind="ExternalOutput")

    relu = mybir.ActivationFunctionType.Relu
    ident = mybir.ActivationFunctionType.Identity
    cspans = _chunks(C)

    with tile.TileContext(nc) as tc:
        with (
            tc.tile_pool(name="sb", bufs=1) as sb,
            tc.tile_pool(name="ps", bufs=8, space="PSUM") as ps,
        ):
            # halves: lets layer-1 matmuls for k=0..1 start at half-load
            w1_t = [sb.tile([P, 2, H], mm_dt, tag=f"w1_{k}", name=f"w1_{k}") for k in range(2)]
            xe_t = [sb.tile([P, 2, C], mm_dt, tag=f"xe_{k}", name=f"xe_{k}") for k in range(2)]
            w2_t = [sb.tile([P, 4, D], mm_dt, tag=f"w2_{k}", name=f"w2_{k}") for k in range(2)]
            b12_t = sb.tile([P, KH + KD], f32)
            hT_t = [sb.tile([P, C], mm_dt, tag=f"h_{k}", name=f"h_{k}") for k in range(KH)]
            out_t = sb.tile([P, KD, C], f32)

            def w1s(k):  # [P, 128] lhsT slice for contraction tile k
                return w1_t[k // 2][:, k % 2]

            def xes(k):
                return xe_t[k // 2][:, k % 2]

            def w2s(k):
                return w2_t[k // 4][:, k % 4]

            # Three concurrent DMA streams (SP ring / ACT ring / SWDGE),
            # two halves per tensor so consumers start at half-load.
            w1_r = w1_d.ap().rearrange("p (ko h) -> p ko h", h=H)
            xe_r = xeT_d.ap().rearrange("p (ko c) -> p ko c", c=C)
            w2_r = w2_d.ap().rearrange("p (ko dd) -> p ko dd", dd=D)
            for k in range(2):
                nc.sync.dma_start(w1_t[k][:], w1_r[:, 2 * k : 2 * k + 2])
                nc.scalar.dma_start(xe_t[k][:], xe_r[:, 2 * k : 2 * k + 2])
            nc.scalar.dma_start(b12_t[:], b12_d.ap())
            for k in range(2):
                nc.gpsimd.dma_start(w2_t[k][:], w2_r[:, 4 * k : 4 * k + 4])

            # layer 1: hT[hi] = relu(sum_k W1[k,hi]^T @ xeT[k] + b1[hi])
            for hi in range(KH):
                for c0, cw in cspans:
                    pt = ps.tile([P, cw], f32, tag="ps")
                    for k in range(KD):
                        nc.tensor.matmul(
                            pt[:],
                            w1s(k)[:, hi * P : (hi + 1) * P],
                            xes(k)[:, c0 : c0 + cw],
                            start=(k == 0),
                            stop=(k == KD - 1),
                        )
                    nc.scalar.activation(
                        hT_t[hi][:, c0 : c0 + cw], pt[:], relu, bias=b12_t[:, hi : hi + 1]
                    )

            # layer 2: outT[di] = sum_k W2[k,di]^T @ hT[k] + b2[di]
            for di in range(KD):
                for c0, cw in cspans:
                    pt = ps.tile([P, cw], f32, tag="ps")
                    for k in range(KH):
                        nc.tensor.matmul(
                            pt[:],
                            w2s(k)[:, di * P : (di + 1) * P],
                            hT_t[k][:, c0 : c0 + cw],
                            start=(k == 0),
                            stop=(k == KH - 1),
                        )
                    nc.scalar.activation(
                        out_t[:, di, c0 : c0 + cw],
                        pt[:],
                        ident,
                        bias=b12_t[:, KH + di : KH + di + 1],
                    )

            # single store on the ACT ring: producer (activation) is the
            # same engine, so the store needs no semaphores
            nc.scalar.dma_start(
                out_d.ap().rearrange("p (ko c) -> p ko c", c=C), out_t[:]
            )

    _split_multi_waits(nc, mybir)
    return nc


def _split_multi_waits(nc, mybir):
    """This walrus build allows only one sync-wait per engine instruction.
    Split any multi-wait instruction: hoist all but the last wait onto
    single-wait NoOps inserted just before it on the same engine."""
    n = 0
    for f in nc.m.functions:
        for blk in f.blocks:
            insts = blk.instructions
            i = 0
            while i < len(insts):
                inst = insts[i]
                si = inst.sync_info
                if si is not None and len(si.on_wait) > 1:
                    waits = list(si.on_wait)
                    for j, w in enumerate(waits[:-1]):
                        nop = mybir.InstNoOp(
                            name=f"ant-waitsplit-{n}",
                            engine=inst.engine,
                            ins=[],
                            outs=[],
                            sync_info=mybir.SyncInfo(on_wait=[w], on_update=[]),
                        )
                        n += 1
                        insts.insert(i, nop)
                        i += 1
                    inst.sync_info = mybir.SyncInfo(
                        on_wait=[waits[-1]], on_update=list(si.on_update)
                    )
                i += 1


def _get_nc(C: int, dtype_mode: str):
    key = (C, dtype_mode)
    if key not in _nc_cache:
        _nc_cache[key] = _build(C, dtype_mode)
    return _nc_cache[key]


def _route(xf: np.ndarray, mask_f: np.ndarray, w_gate: np.ndarray):
    """Top-2 gating on host. Returns per-expert (positions, gate values)."""
    N = xf.shape[0]
    logits = xf @ w_gate  # [N, E] f32
    rows = np.arange(N)
    i1 = np.argmax(logits, axis=1)
    v1 = logits[rows, i1]
    l2 = logits.copy()
    l2[rows, i1] = -np.inf
    i2 = np.argmax(l2, axis=1)
    v2 = l2[rows, i2]
    # softmax over the two top values (v1 >= v2)
    e2 = np.exp(v2 - v1)
    s = 1.0 + e2
    g1 = (1.0 / s).astype(np.float32)
    g2 = (e2 / s).astype(np.float32)
    active = mask_f != 0
    pos, gv = [], []
    for e in range(E):
        s1 = (i1 == e) & active
        s2 = (i2 == e) & active
        p = np.concatenate([np.nonzero(s1)[0], np.nonzero(s2)[0]])
        g = np.concatenate([g1[s1], g2[s2]])
        pos.append(p)
        gv.append(g)
    return pos, gv


def kernel(x, mask, w_gate, W1, b1, W2, b2, dtype_mode="f32r", trace=False):
    global LAST_RESULTS
    import ml_dtypes
    from concourse.bass_utils import run_bass_kernel_spmd

    in_np_dt = ml_dtypes.bfloat16 if dtype_mode == "bf16" else np.float32

    x = np.asarray(x, dtype=np.float32)
    mask_f = np.asarray(mask).reshape(-1)
    w_gate = np.asarray(w_gate, dtype=np.float32)
    W1 = np.asarray(W1, dtype=np.float32)
    b1 = np.asarray(b1, dtype=np.float32)
    W2 = np.asarray(W2, dtype=np.float32)
    b2 = np.asarray(b2, dtype=np.float32)

    xf = x.reshape(-1, D)
    pos, gv = _route(xf, mask_f, w_gate)
    maxc = max(len(p) for p in pos)
    C = max(C_DEFAULT, ((maxc + P - 1) // P) * P)

    nc = _get_nc(C, dtype_mode)

    in_maps = []
    for e in range(E):
        # pre-tiled [P, ktiles*free] layouts (partition-major)
        xeT = np.zeros((P, (D // P) * C), dtype=in_np_dt)
        n_e = len(pos[e])
        ge = xf[pos[e]].T.astype(in_np_dt)  # [D, n_e]
        for k in range(D // P):
            xeT[:, k * C : k * C + n_e] = ge[k * P : (k + 1) * P]
        w1t = W1[e].astype(in_np_dt).reshape(D // P, P, H).transpose(1, 0, 2)
        w2t = W2[e].astype(in_np_dt).reshape(H // P, P, D).transpose(1, 0, 2)
        b12 = np.concatenate(
            [b1[e].reshape(H // P, P).T, b2[e].reshape(D // P, P).T], axis=1
        )
        in_maps.append(
            {
                "xeT": xeT,
                "w1": np.ascontiguousarray(w1t.reshape(P, (D // P) * H)),
                "w2": np.ascontiguousarray(w2t.reshape(P, (H // P) * D)),
                "b12c": np.ascontiguousarray(b12),
            }
        )

    res = run_bass_kernel_spmd(nc, in_maps, core_ids=list(range(N_CORES)), trace=trace)
    LAST_RESULTS = res

    y = xf.copy()
    for e in range(E):
        n_e = len(pos[e])
        if n_e:
            o = res.results[e]["outT"].reshape(P, D // P, C)
            o = o.transpose(1, 0, 2).reshape(D, C)  # row k*P+p = outT row
            y[pos[e]] += gv[e][:, None] * o[:, :n_e].T
    return y.reshape(B, S, D)
